# revision 46
# baseline (speedup 1.0000x reference)
"""Self-contained Trainium2 Bass kernel for nn_BuildSubGraph_32615981645853
(MAGNA graph-attention + per-user batch-norm pooling), SPMD over 8 NeuronCores.

Sharding: graph nodes (3000, padded to 3072) are sharded 8-way (384 rows/core).
Each core computes its rows of the masked-softmax attention matrix E and the
PPR-style diffusion hops; the full diffusion state z is re-assembled per hop
with AllGathers.  Optimizations over the plain per-hop-AllGather schedule:
  * a tiny warmup AllGather at kernel start absorbs the one-time CC-core
    startup latency and the cross-core launch skew under initial compute,
  * collective-receive DMAs are issued from the scalar queue (in 3 chunks)
    so their completion waits never block the sync queue that issues sends,
  * the layer-boundary AllGather ships z already transposed so the full
    h^T for layer 2 is assembled with one strided DMA + add (no 24-chunk
    PE-transpose pass),
  * the end stage uses a ReduceScatter instead of AllReduce: each core
    finalizes only its 32 users and the host concatenates the 8 slices;
    S is broadcast across user partitions with a K=1 matmul, and the Sqrt
    activation table is preloaded under the ReduceScatter.

The per-user gather/batch-norm/pooling stage collapses algebraically to
    out[b,g,h] = gamma[h]*inv[b,h]*(S[g,h] - W*mean[b,h]) + beta[h]*W + bp
where
    S[g,h]    = sum_n softmax_g(graph@Wg)[n,g] * Wp[n] * graph[n,h]
    mean[b,h] = (1/200) * sum_{l: cate[b,l]!=0} graph[cate[b,l],h]
    E2[b,h]   = (1/200) * sum_{l: cate[b,l]!=0} s2[cate[b,l]] * graph[.,h]^2
    var       = E2 - mean^2,  inv = 1/sqrt(var+eps),  W = sum_n Wp[n]
(using sum_g softmax = 1 and sum_g softmax^2 = s2).  The user gather becomes a
host-built count matrix so mean/E2 are tiny matmuls contracted over nodes,
sharded over nodes and finished with the ReduceScatter.
"""

import numpy as np
import ml_dtypes

import concourse.bacc as bacc
import concourse.mybir as mybir
import concourse.tile as tile
from concourse.bass import ts
from concourse.masks import make_identity
from concourse.bass_utils import run_bass_kernel_spmd

BF16 = ml_dtypes.bfloat16

NC = 8
N = 3072
C = N // 128  # 24
OWN = N // NC  # 384
OT = OWN // 128  # 3
H = 64
B = 256
BP = B // NC  # 32 users finalized per core
G = 4
L = 50
NUM_CATES = 3000
ALPHA = 0.15
EPS = 1e-5
SCALE = 0.125

F32 = mybir.dt.float32
BF = mybir.dt.bfloat16
RG = [list(range(NC))]

# jc chunk order: the 8 "A" chunks (it-tile 0 of each core) first, so the
# matmuls depending only on the early half-AllGather can start first.
A_JCS = [3 * c for c in range(NC)]
B_JCS = [3 * c + t for c in range(NC) for t in (1, 2)]
ORD_JCS = A_JCS + B_JCS


def _build_nc(W_scalar: float):
    nc = bacc.Bacc("TRN2", target_bir_lowering=False, debug=False, num_devices=NC)

    i_hT0 = nc.dram_tensor("hT0_bf", [H, N], BF, kind="ExternalInput")
    i_hT0o = nc.dram_tensor("hT0_own", [H, OWN], BF, kind="ExternalInput")
    i_embo = nc.dram_tensor("emb_own", [OWN, H], F32, kind="ExternalInput")
    i_adjT = nc.dram_tensor("adjT_own", [N, OWN], BF, kind="ExternalInput")
    i_W = [
        nc.dram_tensor(nm, [H, H], BF, kind="ExternalInput")
        for nm in ("Wq1", "Wk1", "Wv1", "Wq2", "Wk2", "Wv2")
    ]
    i_Wg = nc.dram_tensor("Wg_ext", [H + 1, G], F32, kind="ExternalInput")
    i_Wpo = nc.dram_tensor("Wp_own", [OWN, 1], F32, kind="ExternalInput")
    i_cnt = nc.dram_tensor("countsT_own", [OWN, B], F32, kind="ExternalInput")
    i_gam = nc.dram_tensor("gamma_row", [1, H], F32, kind="ExternalInput")
    i_bet = nc.dram_tensor("beta_eff_row", [1, H], F32, kind="ExternalInput")
    o_out = nc.dram_tensor("out", [BP, G, H], F32, kind="ExternalOutput")

    with tile.TileContext(nc) as tc:
        with (
            tc.tile_pool(name="const", bufs=1) as constp,
            tc.tile_pool(name="big", bufs=1) as bigp,
            tc.tile_pool(name="lay", bufs=2) as layp,
            tc.tile_pool(name="own", bufs=2) as ownp,
            tc.tile_pool(name="zpool", bufs=2) as zp,
            tc.tile_pool(name="sm", bufs=2) as smp,
            tc.tile_pool(name="psA", bufs=3, space="PSUM") as pstp,
            tc.tile_pool(name="psB", bufs=2, space="PSUM") as phopp,
            tc.tile_pool(name="psC", bufs=1, space="PSUM") as pqp,
            tc.tile_pool(name="psD", bufs=2, space="PSUM") as pmp,
            tc.tile_pool(name="dram", bufs=1, space="DRAM") as dramp,
        ):
            # ---- warmup collective: hides one-time CC startup latency ----
            wu = constp.tile([8, 16], F32)
            nc.vector.memset(wu[:], 0.0)
            wu_in = dramp.tile([8, 16], F32)
            wu_out = dramp.tile([64, 16], F32, addr_space="Shared")
            nc.sync.dma_start(wu_in[:], wu[:])
            nc.gpsimd.collective_compute(
                "AllGather",
                mybir.AluOpType.bypass,
                replica_groups=RG,
                ins=[wu_in.opt()],
                outs=[wu_out.opt()],
            )

            idf = constp.tile([128, 128], F32)
            make_identity(nc, idf[:])
            idb = constp.tile([128, 128], BF)
            nc.vector.tensor_copy(idb[:], idf[:])
            eps32 = constp.tile([BP, 1], F32)
            nc.vector.memset(eps32[:], EPS)
            ones1 = constp.tile([1, BP], F32)
            nc.vector.memset(ones1[:], 1.0)

            # ---- persistent loads ----
            hT0 = constp.tile([H, N], BF)
            nc.sync.dma_start(hT0[:], i_hT0.ap())
            hT0o = constp.tile([H, OWN], BF)
            nc.sync.dma_start(hT0o[:], i_hT0o.ap())
            Wsb = []
            for t in i_W:
                w = constp.tile([H, H], BF, name=f"w_{t.name}")
                nc.sync.dma_start(w[:], t.ap())
                Wsb.append(w)
            adjT = bigp.tile([128, C, OWN], BF)
            for qq in range(4):
                nc.sync.dma_start(
                    adjT[:, qq * 6 : (qq + 1) * 6, :],
                    i_adjT.ap().rearrange("(c p) i -> p c i", p=128)[
                        :, qq * 6 : (qq + 1) * 6, :
                    ],
                )
            # issue the remaining loads on the vector queue so they don't
            # serialize behind the big adjT transfer on the sync queue
            gam = constp.tile([BP, H], F32)
            nc.gpsimd.dma_start(gam[:], i_gam.ap()[0:1, :].partition_broadcast(BP))
            bet = constp.tile([BP, H], F32)
            nc.gpsimd.dma_start(bet[:], i_bet.ap()[0:1, :].partition_broadcast(BP))
            h0o = ownp.tile([128, OT, H], F32, tag="resid")
            nc.gpsimd.dma_start(
                h0o[:], i_embo.ap().rearrange("(t p) f -> p t f", p=128)
            )
            # end-stage constants, loaded early so they hide under compute
            Wgsb = constp.tile([H + 1, G], F32)
            nc.gpsimd.dma_start(Wgsb[:], i_Wg.ap())
            wpsb = constp.tile([128, OT], F32)
            nc.gpsimd.dma_start(
                wpsb[:][:, :, None],
                i_Wpo.ap().rearrange("(t p) f -> p t f", p=128),
            )
            cnt = bigp.tile([128, OT, B], F32)
            nc.gpsimd.dma_start(
                cnt[:], i_cnt.ap().rearrange("(t p) b -> p t b", p=128)
            )

            E = bigp.tile([128, C, OWN], BF)

            hT, hTo, res = hT0, hT0o, h0o
            hfinal = h0o
            for lay in range(2):
                Wq, Wk, Wv = Wsb[3 * lay : 3 * lay + 3]
                # qT_own [H, OWN]
                pq0 = pqp.tile([H, 512], F32, tag="pq", name=f"pq0_{lay}")
                nc.tensor.matmul(pq0[:, :OWN], Wq[:], hTo[:], start=True, stop=True)
                qTo = layp.tile([H, OWN], BF, tag="qto")
                nc.scalar.copy(qTo[:], pq0[:, :OWN])
                # kT full [H, N]; v full (natural) + ones column (gives the
                # row-sum d in hop 0).  Layer 2 uses linearity in h1=h0+z4:
                # Wk2@h0 and h0@Wv2 were precomputed during layer-1's
                # AllGather stalls, so only the z4 terms gate on the
                # boundary collective (no hT1 materialization).
                kT = layp.tile([H, N], BF, tag="kt")
                vsb = layp.tile([128, C, H + 1], BF, tag="v")
                nc.gpsimd.memset(vsb[:, :, H], 1.0)
                if lay == 0:
                    for bb in range(N // 512):
                        pk = pqp.tile([H, 512], F32, tag="pq", name=f"pk{lay}_{bb}")
                        nc.tensor.matmul(
                            pk[:], Wk[:], hT[:, ts(bb, 512)], start=True, stop=True
                        )
                        nc.vector.tensor_copy(kT[:, ts(bb, 512)], pk[:])
                    for jc in range(C):
                        pv = pmp.tile([128, H], F32, tag="pm", name=f"pv{lay}_{jc}")
                        nc.tensor.matmul(
                            pv[:], hT[:, ts(jc, 128)], Wv[:], start=True, stop=True
                        )
                        nc.vector.tensor_copy(vsb[:, jc, :H], pv[:])
                else:
                    # half A covers global columns 0:1536, half B the rest
                    zT3A = zTf_sav[0][:].rearrange("h c i -> h (c i)")
                    zT3B = zTf_sav[1][:].rearrange("h c i -> h (c i)")
                    for bb in range(N // 512):
                        zT3 = zT3A if bb < 3 else zT3B
                        off = ts(bb, 512) if bb < 3 else ts(bb - 3, 512)
                        pk = pqp.tile([H, 512], F32, tag="pq", name=f"pk{lay}_{bb}")
                        nc.tensor.matmul(
                            pk[:], Wk[:], zT3[:, off], start=True, stop=True
                        )
                        nc.vector.tensor_add(
                            kT[:, ts(bb, 512)], pk[:], kT0sb[:, ts(bb, 512)]
                        )
                    for jc in range(C):
                        zT3 = zT3A if jc < 12 else zT3B
                        off = ts(jc, 128) if jc < 12 else ts(jc - 12, 128)
                        pv = pmp.tile([128, H], F32, tag="pm", name=f"pv{lay}_{jc}")
                        nc.tensor.matmul(
                            pv[:], zT3[:, off], Wv[:], start=True, stop=True
                        )
                        nc.vector.tensor_add(
                            vsb[:, jc, :H], pv[:], v0sb[:, jc, :]
                        )
                # alpha*v for own rows (f32)
                avo = ownp.tile([128, OT, H], F32, tag="avo")
                for it in range(OT):
                    pv = pmp.tile([128, H], F32, tag="pm", name=f"pvo{lay}_{it}")
                    nc.tensor.matmul(
                        pv[:], hTo[:, ts(it, 128)], Wv[:], start=True, stop=True
                    )
                    nc.scalar.mul(avo[:, it, :], pv[:], ALPHA)

                # scores^T (own cols), exp, mask -> E
                for jc in range(C):
                    pst = pstp.tile([128, OWN], F32, tag="pst", name=f"pst{lay}_{jc}")
                    nc.tensor.matmul(
                        pst[:], kT[:, ts(jc, 128)], qTo[:], start=True, stop=True
                    )
                    nc.scalar.activation(
                        E[:, jc, :],
                        pst[:],
                        mybir.ActivationFunctionType.Exp,
                        scale=SCALE,
                    )
                    nc.vector.tensor_mul(E[:, jc, :], E[:, jc, :], adjT[:, jc, :])

                # diffusion hops
                wsc = ownp.tile([128, OT], F32, tag="wsc")
                zprev = None
                for hop in range(4):
                    width = H + 1 if hop == 0 else H
                    last_hop = hop == 3
                    boundary = last_hop and lay == 0

                    znew = None
                    if last_hop:
                        znew = ownp.tile(
                            [128, OT, H], F32, tag="znew", name=f"zn{lay}{hop}"
                        )
                    zbfo = ownp.tile(
                        [128, OT, H], BF, tag="zbfo", name=f"zb{lay}{hop}"
                    )
                    for it in range(OT):
                        ph = phopp.tile(
                            [128, H + 1], F32, tag="ph", name=f"ph{lay}{hop}{it}"
                        )
                        if hop > 0:
                            # fold the +alpha*v term into the accumulation
                            # (avo_div = alpha*v/wsc, added via identity
                            # matmul) so one vector op finishes the hop
                            nc.tensor.matmul(
                                ph[:, :H],
                                idb[:],
                                avo_div[:, it, :],
                                start=True,
                                stop=False,
                            )
                        for jc in range(C):
                            if hop == 0:
                                rhs = vsb[:, jc, :]
                            else:
                                rhs = zprev[jc // 8][:, jc % 8, :]
                            nc.tensor.matmul(
                                ph[:, :width],
                                E[:, jc, ts(it, 128)],
                                rhs,
                                start=(jc == 0 and hop == 0),
                                stop=(jc == C - 1),
                            )
                        if hop == 0:
                            nc.vector.tensor_scalar_mul(
                                wsc[:, it : it + 1],
                                ph[:, H : H + 1],
                                1.0 / (1.0 - ALPHA),
                            )
                            nc.vector.reciprocal(
                                wsc[:, it : it + 1], wsc[:, it : it + 1]
                            )
                        if last_hop:
                            # keep f32 for the residual add
                            nc.vector.tensor_scalar(
                                znew[:, it, :],
                                ph[:, :H],
                                wsc[:, it : it + 1],
                                None,
                                mybir.AluOpType.mult,
                            )
                        else:
                            if hop > 0 and it == 1:
                                # middle tile on the scalar engine: the three
                                # finishing scales run on two queues
                                nc.scalar.activation(
                                    zbfo[:, it, :],
                                    ph[:, :H],
                                    mybir.ActivationFunctionType.Copy,
                                    scale=wsc[:, it : it + 1],
                                )
                            else:
                                nc.vector.tensor_scalar(
                                    zbfo[:, it, :],
                                    ph[:, :H],
                                    wsc[:, it : it + 1],
                                    None,
                                    mybir.AluOpType.mult,
                                )
                            if hop == 0:
                                nc.vector.tensor_add(
                                    zbfo[:, it, :], zbfo[:, it, :], avo[:, it, :]
                                )
                    if hop == 0:
                        # avo_div = alpha*v / wsc for the later hops' folded add
                        rec = ownp.tile([128, OT], F32, tag="rec")
                        nc.vector.reciprocal(rec[:], wsc[:])
                        avo_div = ownp.tile([128, OT, H], BF, tag="avd")
                        nc.vector.tensor_mul(
                            avo_div[:],
                            avo[:],
                            rec[:][:, :, None].broadcast_to([128, OT, H]),
                        )

                    if not last_hop:
                        ccin = dramp.tile([OWN, H], BF, tag="ccin", bufs=2)
                        ccout = dramp.tile(
                            [N, H], BF, addr_space="Shared", tag="ccout", bufs=2
                        )
                        nc.sync.dma_start(
                            ccin[:].rearrange("(t p) f -> p t f", p=128), zbfo[:]
                        )
                        nc.gpsimd.collective_compute(
                            "AllGather",
                            mybir.AluOpType.bypass,
                            replica_groups=RG,
                            ins=[ccin.opt()],
                            outs=[ccout.opt()],
                        )
                        # receive in 3 chunks, one per DMA-capable queue, so
                        # all issues fire in parallel right after the
                        # collective completes and the first matmuls start
                        # before the tail lands (sync is safe: the next send
                        # depends on this data anyway)
                        zprev = []
                        for q, eng in enumerate((nc.sync, nc.scalar, nc.gpsimd)):
                            znq = zp.tile(
                                [128, 8, H], BF, tag=f"z{q}", name=f"z{lay}{hop}{q}"
                            )
                            eng.dma_start(
                                znq[:],
                                ccout[:].rearrange("(c p) f -> p c f", p=128)[
                                    :, 8 * q : 8 * (q + 1), :
                                ],
                            )
                            zprev.append(znq)
                        if lay == 0 and hop == 0:
                            # layer-2 h0 contributions, computed while the
                            # PE would otherwise idle in AllGather stalls
                            Wk2s, Wv2s = Wsb[4], Wsb[5]
                            kT0sb = layp.tile([H, N], BF, tag="kt0")
                            for bb in range(N // 512):
                                pk = pqp.tile(
                                    [H, 512], F32, tag="pq", name=f"pk0p_{bb}"
                                )
                                nc.tensor.matmul(
                                    pk[:],
                                    Wk2s[:],
                                    hT0[:, ts(bb, 512)],
                                    start=True,
                                    stop=True,
                                )
                                nc.vector.tensor_copy(
                                    kT0sb[:, ts(bb, 512)], pk[:]
                                )
                            v0sb = layp.tile([128, C, H], BF, tag="v00")
                            for jc in range(C):
                                pv = pmp.tile(
                                    [128, H], F32, tag="pm", name=f"pv0p_{jc}"
                                )
                                nc.tensor.matmul(
                                    pv[:],
                                    hT0[:, ts(jc, 128)],
                                    Wv2s[:],
                                    start=True,
                                    stop=True,
                                )
                                nc.vector.tensor_copy(v0sb[:, jc, :], pv[:])
                    else:
                        hres = ownp.tile(
                            [128, OT, H], F32, tag="resid", name=f"hres{lay}"
                        )
                        hfinal = hres
                        if boundary:
                            # z^T own + AllGather first (critical path)
                            zTo = layp.tile([H, OWN], BF, tag="zto")
                            for it in range(OT):
                                ptq = pmp.tile(
                                    [H, 128], BF, tag="pm", name=f"ptq{lay}{it}"
                                )
                                nc.vector.tensor_copy(zbfo[:, it, :], znew[:, it, :])
                                nc.tensor.transpose(ptq[:], zbfo[:, it, :], idb[:])
                                nc.vector.tensor_copy(zTo[:, ts(it, 128)], ptq[:])
                            ccinT = dramp.tile([H, OWN], BF, tag="ccT")
                            ccoutT = dramp.tile(
                                [NC * H, OWN], BF, addr_space="Shared", tag="ccoT"
                            )
                            nc.sync.dma_start(ccinT[:], zTo[:])
                            nc.gpsimd.collective_compute(
                                "AllGather",
                                mybir.AluOpType.bypass,
                                replica_groups=RG,
                                ins=[ccinT.opt()],
                                outs=[ccoutT.opt()],
                            )
                            zTfA = layp.tile([H, NC // 2, OWN], BF, tag="ztfa")
                            nc.scalar.dma_start(
                                zTfA[:],
                                ccoutT[:].rearrange("(c h) i -> h c i", h=H)[
                                    :, :4, :
                                ],
                            )
                            zTfB = layp.tile([H, NC // 2, OWN], BF, tag="ztfb")
                            nc.sync.dma_start(
                                zTfB[:],
                                ccoutT[:].rearrange("(c h) i -> h c i", h=H)[
                                    :, 4:, :
                                ],
                            )
                            zTf_sav = (zTfA, zTfB)
                            hT1o = layp.tile([H, OWN], BF, tag="ht1o")
                            nc.vector.tensor_add(hT1o[:], hT0o[:], zTo[:])
                            for it in range(OT):
                                nc.vector.tensor_add(
                                    hres[:, it, :], res[:, it, :], znew[:, it, :]
                                )
                            hT, hTo, res = None, hT1o, hres
                        else:
                            for it in range(OT):
                                nc.vector.tensor_add(
                                    hres[:, it, :], res[:, it, :], znew[:, it, :]
                                )

            # ================= end stage =================
            graph = hfinal
            gTe = layp.tile([H + 1, OWN], F32, tag="gte")
            nc.vector.memset(gTe[H : H + 1, :], 1.0)
            for it in range(OT):
                pt = pmp.tile([H, 128], F32, tag="pm", name=f"gt{it}")
                nc.tensor.transpose(pt[:], graph[:, it, :], idf[:])
                nc.vector.tensor_copy(gTe[:H, ts(it, 128)], pt[:])
            sc = smp.tile([128, OT, G], F32, tag="sc", bufs=1)
            for it in range(OT):
                pc = pmp.tile([128, G], F32, tag="pm", name=f"pc{it}")
                nc.tensor.matmul(
                    pc[:], gTe[:, ts(it, 128)], Wgsb[:], start=True, stop=True
                )
                nc.scalar.activation(
                    sc[:, it, :], pc[:], mybir.ActivationFunctionType.Exp
                )
            s2 = smp.tile([128, OT, 1], F32, tag="s2", bufs=1)
            wsc2 = smp.tile([128, OT, G], F32, tag="wsc2", bufs=1)
            g2 = smp.tile([128, OT, H], F32, tag="g2", bufs=1)
            c2 = bigp.tile([128, OT, B], F32)
            rs = smp.tile([128, OT, 1], F32, tag="rs")
            nc.vector.tensor_reduce(
                rs[:], sc[:], axis=mybir.AxisListType.X, op=mybir.AluOpType.add
            )
            nc.vector.reciprocal(rs[:], rs[:])
            nc.vector.tensor_mul(
                sc[:], sc[:], rs[:].broadcast_to([128, OT, G])
            )
            sq = smp.tile([128, OT, G], F32, tag="sq")
            nc.vector.tensor_mul(sq[:], sc[:], sc[:])
            nc.vector.tensor_reduce(
                s2[:], sq[:], axis=mybir.AxisListType.X, op=mybir.AluOpType.add
            )
            nc.vector.tensor_mul(
                wsc2[:], sc[:], wpsb[:][:, :, None].broadcast_to([128, OT, G])
            )
            nc.vector.tensor_mul(g2[:], graph[:], graph[:])
            nc.vector.tensor_mul(
                c2[:], cnt[:], s2[:].broadcast_to([128, OT, B])
            )


            # partial sums packed for the ReduceScatter:
            # segment c (68 rows) = [mean of users 32c..+31, E2 of same, S]
            SEG = 2 * BP + G
            arin = dramp.tile([NC * SEG, H], F32)
            arout = dramp.tile([SEG, H], F32)

            # mean first: it only needs cnt+graph, so its DMAs fly while the
            # softmax/s2 chain that feeds E2 and S is still running
            for bt in range(2):
                smean = smp.tile([128, H], F32, tag="smean", bufs=2, name=f"sm{bt}")
                pmean = pmp.tile([128, H], F32, tag="pm", name=f"pmean{bt}")
                for it in range(OT):
                    nc.tensor.matmul(
                        pmean[:],
                        cnt[:, it, ts(bt, 128)],
                        graph[:, it, :],
                        start=(it == 0),
                        stop=(it == OT - 1),
                    )
                nc.vector.tensor_copy(smean[:], pmean[:])
                meng = nc.sync if bt == 0 else nc.scalar
                for q in range(4):
                    c = 4 * bt + q
                    meng.dma_start(
                        arin[SEG * c : SEG * c + BP, :],
                        smean[BP * q : BP * (q + 1), :],
                    )

            sS = smp.tile([G, H], F32, tag="sS", bufs=1)
            pS = pmp.tile([G, H], F32, tag="pm", name="pS")
            for it in range(OT):
                nc.tensor.matmul(
                    pS[:],
                    wsc2[:, it, :],
                    graph[:, it, :],
                    start=(it == 0),
                    stop=(it == OT - 1),
                )
            nc.vector.tensor_copy(sS[:], pS[:])
            for c in range(NC):
                nc.gpsimd.dma_start(
                    arin[SEG * c + 2 * BP : SEG * (c + 1), :],
                    sS[:],
                )
            for bt in range(2):
                sE2 = smp.tile([128, H], F32, tag="sE2", bufs=2, name=f"se{bt}")
                pE2 = pmp.tile([128, H], F32, tag="pm", name=f"pE2{bt}")
                for it in range(OT):
                    nc.tensor.matmul(
                        pE2[:],
                        c2[:, it, ts(bt, 128)],
                        g2[:, it, :],
                        start=(it == 0),
                        stop=(it == OT - 1),
                    )
                nc.vector.tensor_copy(sE2[:], pE2[:])
                meng = nc.sync if bt == 0 else nc.scalar
                for q in range(4):
                    c = 4 * bt + q
                    meng.dma_start(
                        arin[SEG * c + BP : SEG * c + 2 * BP, :],
                        sE2[BP * q : BP * (q + 1), :],
                    )

            # preload the Sqrt activation table while the ReduceScatter runs;
            # must come after the last scalar Copy/Exp so the table isn't
            # thrashed before sd uses it (DMA out so DCE keeps the op)
            sq_warm = smp.tile([1, 1], F32, tag="sqw", bufs=1)
            nc.scalar.activation(
                sq_warm[:], eps32[:1, :], mybir.ActivationFunctionType.Sqrt
            )
            sq_sink = dramp.tile([1, 1], F32)
            nc.sync.dma_start(sq_sink[:], sq_warm[:])

            nc.gpsimd.collective_compute(
                "ReduceScatter",
                mybir.AluOpType.add,
                replica_groups=RG,
                ins=[arin.opt()],
                outs=[arout.opt()],
            )

            # finalize this core's 32 users
            mE = smp.tile([BP, 2, H], F32, tag="mE", bufs=1)
            nc.scalar.dma_start(mE[:, 0, :], arout[:BP, :])
            nc.sync.dma_start(mE[:, 1, :], arout[BP : 2 * BP, :])
            # broadcast S to the 32 user partitions with a K=1 matmul
            Sfl = smp.tile([1, G * H], F32, tag="Sfl", bufs=1)
            nc.gpsimd.dma_start(
                Sfl[:],
                arout[2 * BP : SEG, :].rearrange("(o g) h -> o (g h)", o=1),
            )
            pSb = pmp.tile([BP, G * H], F32, tag="pm", name="pSb")
            nc.tensor.matmul(pSb[:], ones1[:], Sfl[:], start=True, stop=True)
            var = smp.tile([BP, H], F32, tag="var", bufs=1)
            nc.vector.tensor_mul(var[:], mE[:, 0, :], mE[:, 0, :])
            nc.vector.tensor_sub(var[:], mE[:, 1, :], var[:])
            sd = smp.tile([BP, H], F32, tag="sd", bufs=1)
            nc.scalar.activation(
                sd[:],
                var[:],
                mybir.ActivationFunctionType.Sqrt,
                bias=eps32[:],
            )
            Am = smp.tile([BP, H], F32, tag="Am", bufs=1)
            nc.vector.reciprocal(Am[:], sd[:])
            nc.vector.tensor_mul(Am[:], Am[:], gam[:])
            Bc = smp.tile([BP, H], F32, tag="Bc", bufs=1)
            nc.vector.tensor_mul(Bc[:], Am[:], mE[:, 0, :])
            nc.vector.tensor_scalar_mul(Bc[:], Bc[:], -W_scalar)
            nc.vector.tensor_add(Bc[:], Bc[:], bet[:])
            og = smp.tile([BP, G, H], F32, tag="og", bufs=1)
            nc.vector.tensor_mul(
                og[:],
                pSb[:].rearrange("p (g h) -> p g h", g=G),
                Am[:, None, :].broadcast_to([BP, G, H]),
            )
            nc.vector.tensor_add(
                og[:], og[:], Bc[:, None, :].broadcast_to([BP, G, H])
            )
            nc.sync.dma_start(o_out.ap()[:, :, :], og[:])

    nc.compile()
    return nc


def _prep_inputs(inputs):
    cate = np.asarray(inputs["cate_list"])
    adj = np.asarray(inputs["adj"], np.float32)
    emb = np.asarray(inputs["emb"], np.float32)
    Wq = np.asarray(inputs["Wq"], np.float32)
    Wk = np.asarray(inputs["Wk"], np.float32)
    Wv = np.asarray(inputs["Wv"], np.float32)
    Wg = np.asarray(inputs["Wg"], np.float32)
    bg = np.asarray(inputs["bg"], np.float32)
    Wp = np.asarray(inputs["Wp"], np.float32)
    bp = np.asarray(inputs["bp"], np.float32)
    gamma = np.asarray(inputs["gamma"], np.float32)
    beta = np.asarray(inputs["beta"], np.float32)

    adjP = np.zeros((N, N), np.float32)
    adjP[:NUM_CATES, :NUM_CATES] = adj
    idx = np.arange(NUM_CATES, N)
    adjP[idx, idx] = 1.0

    embP = np.zeros((N, H), np.float32)
    embP[:NUM_CATES] = emb
    hT0 = np.ascontiguousarray(embP.T).astype(BF16)

    WpP = np.zeros((N, 1), np.float32)
    WpP[:NUM_CATES] = Wp
    W = float(Wp.sum())
    beta_eff = (beta * W + bp).astype(np.float32).reshape(1, H)
    Wg_ext = np.concatenate([Wg, bg.reshape(1, G)], axis=0).astype(np.float32)

    counts = np.zeros((B, N), np.float32)
    bi = np.repeat(np.arange(B), L)
    ci = cate.reshape(-1).astype(np.int64)
    msk = (ci != 0).astype(np.float32) / float(G * L)
    np.add.at(counts, (bi, ci), msk)
    countsT = np.ascontiguousarray(counts.T)

    in_maps = []
    for c in range(NC):
        sl = slice(c * OWN, (c + 1) * OWN)
        in_maps.append(
            {
                "hT0_bf": hT0,
                "hT0_own": np.ascontiguousarray(hT0[:, sl]),
                "emb_own": np.ascontiguousarray(embP[sl]),
                "adjT_own": np.ascontiguousarray(adjP[sl].T).astype(BF16),
                "Wq1": Wq[0].astype(BF16),
                "Wk1": Wk[0].astype(BF16),
                "Wv1": Wv[0].astype(BF16),
                "Wq2": Wq[1].astype(BF16),
                "Wk2": Wk[1].astype(BF16),
                "Wv2": Wv[1].astype(BF16),
                "Wg_ext": Wg_ext,
                "Wp_own": np.ascontiguousarray(WpP[sl]),
                "countsT_own": np.ascontiguousarray(countsT[sl]),
                "gamma_row": gamma.reshape(1, H).astype(np.float32),
                "beta_eff_row": beta_eff,
            }
        )
    return in_maps, W


def _assemble(results) -> np.ndarray:
    return np.concatenate(
        [np.asarray(r["out"], np.float32) for r in results], axis=0
    )


_NC_CACHE = {}


def kernel(**inputs) -> np.ndarray:
    in_maps, W = _prep_inputs(inputs)
    key = round(W, 10)
    nc = _NC_CACHE.get(key)
    if nc is None:
        nc = _build_nc(W)
        _NC_CACHE[key] = nc
    res = run_bass_kernel_spmd(nc, in_maps, core_ids=list(range(NC)))
    return _assemble(res.results)
